# revision 11
# baseline (speedup 1.0000x reference)
"""DTNNStep Bass kernel for Trainium2 (8 NeuronCores, data-parallel over batch).

Computes, per molecule b:
    dist_h = dist @ W_df + b_df              # [N, N, H]
    atom_h = atom @ W_cf + b_cf              # [N, H]
    gated  = dist_h * atom_h[None, :, :]     # broadcast over i
    out    = tanh(gated @ W_fc)              # mask == 1 in this benchmark
    result = out.sum(axis=1) + atom          # [N, F]

v2 pipeline (per core = 2 molecules):
  1. dist loads are fully-contiguous fp32 DMAs [128p, 16i, 100d] where the
     partition p is a block of 8 consecutive (i,j) pairs -> 6.4KB/partition
     runs (SWDGE on the gpsimd queue, no tiny-descriptor penalty).
  2. fp32 -> bf16 cast into a d-padded [128j, 16i, 128d] tile, split across
     scalar/DVE/gpsimd engines.  Pad col 100 = 1.0 (bias fold), 101.. = 0.
  3. XBAR DMA transpose (SP queue, SBUF->SBUF bf16) -> distT [128d, 16i, 128j].
     No PE transposes, no PSUM->SBUF copy.
  4. mm1 (W_df_aug stationary), gate vs atom_h^T, mm2 (W_fc), all bf16 with
     bf16 PSUM tiles so the element ops run at 2x 16-bit throughput.
  5. tanh (scalar, bf16 out), j-reduce (DVE, bf16), final transpose + add.
"""

import os
import sys

import numpy as np

for _p in ("/opt/trn_rl_repo", os.path.expanduser("~/.axon_site/_ro/trn_rl_repo")):
    if os.path.isdir(_p) and _p not in sys.path:
        sys.path.insert(0, _p)

import concourse.bass as bass
import concourse.tile as tile
from concourse import bacc, mybir
from concourse.bass import ds
from concourse.bass_utils import run_bass_kernel_spmd
from concourse.masks import make_identity

B, N, NF, ND, NH = 16, 128, 64, 100, 64
NCORES = 8
BPC = B // NCORES  # molecules per core

F32 = mybir.dt.float32
BF16 = mybir.dt.bfloat16

NDP = 128  # padded d (col 100 = ones for bias fold, 101..127 = 0)
IL = 16  # i's per load/cast/transpose tile (two 8-i compute groups)
G = 4  # i's per mm half (PSUM free dim = G*N = 512)
NBUF = 3  # buffers for the big staging pools

# cast split: of the IL=16 i-slices per tile, how many go to each engine
CAST_SCALAR = 4
CAST_DVE = 0
CAST_GP = IL - CAST_SCALAR - CAST_DVE


def _emit(tc):
    nc = tc.nc
    dist = nc.dram_tensor("dist", (BPC, N, N, ND), F32, kind="ExternalInput").ap()
    atom = nc.dram_tensor("atom", (BPC, N, NF), F32, kind="ExternalInput").ap()
    w_cf = nc.dram_tensor("w_cf", (NF, NH), F32, kind="ExternalInput").ap()
    w_df = nc.dram_tensor("w_df", (ND, NH), F32, kind="ExternalInput").ap()
    w_fc = nc.dram_tensor("w_fc", (NH, NF), F32, kind="ExternalInput").ap()
    b_cf = nc.dram_tensor("b_cf", (1, NH), F32, kind="ExternalInput").ap()
    b_df = nc.dram_tensor("b_df", (1, NH), F32, kind="ExternalInput").ap()
    out = nc.dram_tensor("out", (BPC, N, NF), F32, kind="ExternalOutput").ap()

    # dist[b] viewed as flat pairs: pair = i*N + j.  A load tile covers
    # IL*N = 2048 pairs; partition p holds the 8 consecutive pairs
    # [base + 8p, base + 8p + 8)  ->  i = 16*t + p//16 .. (one i per 16 p),
    # j = 8*(p%16) + a for a in 0..8.  Wait: with 2048 pairs over 128
    # partitions each partition holds 16 pairs: i = t*16 + p//8, j =
    # 16*(p%8) + a, a in 0..16.  See PAIRS_PER_PART below.
    PAIRS_PER_PART = IL * N // 128  # 16

    with (
        tc.tile_pool(name="consts", bufs=1) as consts,
        tc.tile_pool(name="raw", bufs=NBUF) as rawp,
        tc.tile_pool(name="distT", bufs=NBUF) as dtp,
        tc.tile_pool(name="work", bufs=4) as work,
        tc.tile_pool(name="perb", bufs=2) as perb,
        tc.tile_pool(name="pmm", bufs=2, space="PSUM") as pmm,
        tc.tile_pool(name="pmm2", bufs=2, space="PSUM") as pmm2,
        tc.tile_pool(name="psmall", bufs=1, space="PSUM") as psmall,
    ):
        identity = consts.tile([128, 128], F32)
        make_identity(nc, identity)
        identity_bf = consts.tile([128, 128], BF16)
        make_identity(nc, identity_bf)
        ones_f32 = consts.tile([1, N], F32)
        nc.vector.memset(ones_f32, 1.0)

        # Preload the tanh table set while the first DMAs are in flight.
        warm_tanh = consts.tile([1, 8], F32)
        nc.scalar.activation(warm_tanh, ones_f32[:, :8], mybir.ActivationFunctionType.Tanh)

        # W_df_aug [NDP, NH]: rows 0..99 = W_df, row 100 = b_df, 101.. = 0.
        w_df_aug_f = consts.tile([NDP, NH], F32)
        nc.vector.memset(w_df_aug_f, 0.0)
        nc.sync.dma_start(w_df_aug_f[:ND], w_df)
        nc.sync.dma_start(w_df_aug_f[ND : ND + 1], b_df)
        w_df_aug = consts.tile([NDP, NH], BF16)
        nc.vector.tensor_copy(w_df_aug, w_df_aug_f)

        # W_fc stacked twice vertically so the partition-hi mm2 has its
        # stationary at the same base partition as its rhs.
        w_fc_f = consts.tile([2 * NH, NF], F32)
        nc.sync.dma_start(w_fc_f[:NH], w_fc)
        nc.sync.dma_start(w_fc_f[NH:], w_fc)
        w_fc_bf = consts.tile([2 * NH, NF], BF16)
        nc.vector.tensor_copy(w_fc_bf, w_fc_f)

        # W_cf (and b_cf) duplicated horizontally so atom_hT comes out
        # stacked twice vertically: [2*NH, N].
        w_cf_dup = consts.tile([NF, 2 * NH], F32)
        nc.sync.dma_start(w_cf_dup[:, :NH], w_cf)
        nc.sync.dma_start(w_cf_dup[:, NH:], w_cf)
        bcf_dup = consts.tile([1, 2 * NH], F32)
        nc.sync.dma_start(bcf_dup[:, :NH], b_cf)
        nc.sync.dma_start(bcf_dup[:, NH:], b_cf)

        # Padded bf16 staging tiles [128 j?, IL, NDP].  Managed explicitly so
        # the pad columns (d >= ND) can be initialized exactly once.
        padded_tiles = []
        for k in range(NBUF):
            pt = consts.tile([128, IL, NDP], BF16, tag=f"padded{k}")
            nc.gpsimd.memset(pt[:, :, ND : ND + 1], 1.0)
            nc.gpsimd.memset(pt[:, :, ND + 1 :], 0.0)
            padded_tiles.append(pt)

        # --- per-molecule atom_h^T prep (fp32 PE path, tiny) ---
        atom_bfs = []
        for b in range(BPC):
            atom_in = work.tile([N, NF], F32, tag="atom_in")
            nc.sync.dma_start(atom_in, atom[b])
            atomT_ps = psmall.tile([NF, N], F32, tag="small_ps")
            nc.tensor.transpose(atomT_ps, atom_in, identity)
            atomT = work.tile([NF, N], F32, tag="atomT")
            nc.vector.tensor_copy(atomT, atomT_ps)
            ah_ps = psmall.tile([2 * NH, N], F32, tag="small_ps")
            nc.tensor.matmul(ah_ps, w_cf_dup, atomT, start=True, stop=False)
            nc.tensor.matmul(ah_ps, bcf_dup, ones_f32, start=False, stop=True)
            atom_hT2 = perb.tile([2 * NH, N], BF16, tag="atom_hT2")
            nc.vector.tensor_copy(atom_hT2, ah_ps)
            atom_bfs.append(atom_hT2)

        NT = N // IL  # load tiles per molecule (8)
        NG = IL // (2 * G)  # compute groups per load tile (2)

        for b in range(BPC):
            atom_hT2 = atom_bfs[b]
            # res_pack[(u f), (g q)] accumulates the j-sums; output row
            # i = 8*g + 4*u + q.
            res_pack = perb.tile([2 * NF, 4 * (N // (2 * G))], BF16, tag="res_pack")

            for t in range(NT):
                # 1) contiguous fp32 load: 2048 pairs as [128, 16, 100],
                # 6.4KB per partition, one run.
                raw = rawp.tile([128, PAIRS_PER_PART, ND], BF16, tag="raw")
                src = (
                    dist[b]
                    .rearrange("i j d -> (i j) d")[ds(t * IL * N, IL * N)]
                    .rearrange("(p a) d -> p a d", p=128)
                )
                nc.gpsimd.dma_start(raw, src)

                # 2) cast+pad into padded[j_part? p, a, d] bf16, split across
                # three engines along the a axis.
                padded = padded_tiles[t % NBUF]
                a0 = 0
                for eng, na in (
                    (nc.scalar, CAST_SCALAR),
                    (nc.vector, CAST_DVE),
                    (nc.gpsimd, CAST_GP),
                ):
                    if na:
                        dst = padded[:, ds(a0, na), :ND]
                        srcv = raw[:, ds(a0, na), :]
                        if eng is nc.scalar:
                            eng.copy(dst, srcv)
                        else:
                            eng.tensor_copy(dst, srcv)
                        a0 += na

                # 3) XBAR transpose: in [128 p, (a d)] -> out[d, a, p]
                # out[d, a, p] = padded[p, a, d]; p indexes the 16-pair
                # blocks, d the padded distance channel.
                distT = dtp.tile([NDP, PAIRS_PER_PART, 128], BF16, tag="distT")
                nc.sync.dma_start(distT, padded, transpose=True)

                # distT[d, a, p] = padded[p, a, d]; local pair index within
                # the tile is q = p*16 + a (p outer!), so a [d, (p a)] view
                # over 32 consecutive p's = 512 consecutive pairs = 4 i's
                # with natural j order.
                for g2 in range(NG):
                    # compute group covers 8 i's: i = t*16 + 8*g2 .. +8
                    g = t * NG + g2
                    # moving operands for the two 4-i halves
                    # pairs for half u: local pair range [g2*1024 + u*512,
                    # +512) = local columns c = p*16+a.
                    mhalves = []
                    for u in range(2):
                        c0 = g2 * (2 * G * N) + u * (G * N)
                        # c = p*16 + a: c0..c0+512 spans p in
                        # [c0//16, (c0+512)//16), all a.  512 = 32 p's.
                        p0 = c0 // PAIRS_PER_PART
                        mv = distT[:, :, ds(p0, G * N // PAIRS_PER_PART)].rearrange(
                            "d a p -> d p a"
                        )
                        mhalves.append(mv)

                    out1_ps = pmm.tile([2 * NH, G * N], F32, tag="out1")
                    nc.tensor.matmul(
                        out1_ps[:NH], w_df_aug, mhalves[0], start=True, stop=True
                    )
                    nc.tensor.matmul(
                        out1_ps[NH:], w_df_aug, mhalves[1], start=True, stop=True
                    )

                    # gate with atom_h^T (broadcast over the G i's per half),
                    # all bf16 -> 2x DVE throughput
                    gatedT = work.tile([2 * NH, G * N], BF16, tag="gatedT")
                    nc.vector.tensor_tensor(
                        gatedT.rearrange("h (i j) -> h i j", i=G),
                        out1_ps.rearrange("h (i j) -> h i j", i=G),
                        atom_hT2[:, None, :].to_broadcast((2 * NH, G, N)),
                        mybir.AluOpType.mult,
                    )

                    # mm2: out2^T = W_fc^T @ gatedT, per partition half
                    out2_ps = pmm2.tile([2 * NF, G * N], F32, tag="out2")
                    nc.tensor.matmul(
                        out2_ps[:NF], w_fc_bf[:NH], gatedT[:NH], start=True, stop=True
                    )
                    nc.tensor.matmul(
                        out2_ps[NF:], w_fc_bf[NH:], gatedT[NH:], start=True, stop=True
                    )

                    # tanh (scalar, bf16 in/out) then reduce over j (DVE bf16)
                    tanh_sb = work.tile([2 * NF, G * N], BF16, tag="tanh_sb")
                    nc.scalar.activation(
                        tanh_sb, out2_ps, mybir.ActivationFunctionType.Tanh
                    )
                    with nc.allow_low_precision(
                        reason="128-term bf16 j-sum, tol 2e-2"
                    ):
                        nc.vector.tensor_reduce(
                            res_pack[:, ds(4 * g, G)],
                            tanh_sb.rearrange("f (i j) -> f i j", i=G),
                            axis=mybir.AxisListType.X,
                            op=mybir.AluOpType.add,
                        )

            # --- finalize molecule: out[b] rows i = 8g + 4u + q ---
            atom_v = atom[b].rearrange("(g u q) f -> u g q f", u=2, q=G)
            out_v = out[b].rearrange("(g u q) f -> u g q f", u=2, q=G)
            for u in range(2):
                resT_ps = psmall.tile([N // 2, NF], BF16, tag="small_ps_bf")
                nc.tensor.transpose(
                    resT_ps,
                    res_pack[ds(u * NF, NF)],
                    identity_bf[ds(u * NF, NF), ds(u * NF, N // 2)],
                )
                atom_nat = work.tile([N // 2, NF], F32, tag="atom_nat")
                nc.scalar.dma_start(atom_nat, atom_v[u])
                out_sb = work.tile([N // 2, NF], F32, tag="out_sb")
                nc.vector.tensor_add(out_sb, resT_ps, atom_nat)
                nc.scalar.dma_start(out_v[u], out_sb)


_NC_CACHE = None


def _get_nc():
    global _NC_CACHE
    if _NC_CACHE is None:
        nc = bacc.Bacc("TRN2", target_bir_lowering=False, debug=False)
        with tile.TileContext(nc) as tc:
            _emit(tc)
        nc.compile()
        _NC_CACHE = nc
    return _NC_CACHE


def _numpy_reference(atom, dist, mask, w_cf, w_df, w_fc, b_cf, b_df):
    dist_h = np.einsum("bijd,dh->bijh", dist, w_df) + b_df
    atom_h = np.einsum("bjf,fh->bjh", atom, w_cf) + b_cf
    gated = dist_h * atom_h[:, None, :, :]
    o = np.einsum("bijh,hf->bijf", gated, w_fc)
    o = np.tanh(o * mask[..., None])
    return (o.sum(axis=2) + atom).astype(np.float32)


def run_sharded(inputs, trace=False):
    """Shard over the batch axis, run on 8 cores, gather. Returns (out, results)."""
    atom = np.ascontiguousarray(np.asarray(inputs["atom_features"], np.float32))
    dist = np.ascontiguousarray(np.asarray(inputs["distance_matrix"], np.float32))
    w_cf = np.ascontiguousarray(np.asarray(inputs["W_cf"], np.float32))
    w_df = np.ascontiguousarray(np.asarray(inputs["W_df"], np.float32))
    w_fc = np.ascontiguousarray(np.asarray(inputs["W_fc"], np.float32))
    b_cf = np.asarray(inputs["b_cf"], np.float32).reshape(1, NH)
    b_df = np.asarray(inputs["b_df"], np.float32).reshape(1, NH)

    nc = _get_nc()
    in_maps = []
    for c in range(NCORES):
        sl = slice(c * BPC, (c + 1) * BPC)
        in_maps.append(
            {
                "dist": dist[sl],
                "atom": atom[sl],
                "w_cf": w_cf,
                "w_df": w_df,
                "w_fc": w_fc,
                "b_cf": b_cf,
                "b_df": b_df,
            }
        )
    res = run_bass_kernel_spmd(nc, in_maps, core_ids=list(range(NCORES)), trace=trace)
    out = np.concatenate([res.results[c]["out"] for c in range(NCORES)], axis=0)
    return out, res


def kernel(**inputs) -> np.ndarray:
    mask = np.asarray(inputs["distance_matrix_mask"], np.float32)
    if not np.all(mask == 1.0):
        # The hardware pipeline folds the (always-ones) mask away; keep a
        # correct path for arbitrary masks.
        return _numpy_reference(
            np.asarray(inputs["atom_features"], np.float32),
            np.asarray(inputs["distance_matrix"], np.float32),
            mask,
            np.asarray(inputs["W_cf"], np.float32),
            np.asarray(inputs["W_df"], np.float32),
            np.asarray(inputs["W_fc"], np.float32),
            np.asarray(inputs["b_cf"], np.float32),
            np.asarray(inputs["b_df"], np.float32),
        )
    out, _ = run_sharded(inputs)
    return out
